# revision 11
# baseline (speedup 1.0000x reference)
"""TRN2 Bass kernel for nn_MultiHeadAttn_1580547971654.

Multi-head attention with sigmoid activation (no softmax normalization),
2D key-side mask. query [2,1024,1024], key/value [2,2048,1024],
Wq/Wk/Wv [1024,1024], Wo [1024,1024], NH=16, HD=64.

Sharding (8 cores): data-parallel over batch (2) x tensor-parallel over
head groups (4 groups of 4 heads).  Core (b, g) computes
  partial[b] = sigmoid(scale * (q[b] Wq[:,G]) (k[b] Wk[:,G])^T) ((v[b]*mask) Wv[:,G]) Wo[G,:]
with G = head-group g's 256-wide hidden slice.  Host sums 4 partials per
batch.

Mask compaction: masked klen positions contribute exactly zero
(reference: sigmoid(-1e30) == 0), so the host gathers only unmasked
key/value columns, zero-padded to a multiple of 128.  With the uniform
0/1 mask this halves the klen-side work exactly.

Numerics: fp16 operands everywhere (TRN2 PE does native fp16 multiplies
with fp32 PSUM accumulation), so the only error is rounding tensors to
fp16 (2^-11).  Scale is folded into the sigmoid activation's scale.

Layout: activations are uploaded pre-transposed ([hidden, len]) so all
matmuls contract over the partition axis with no on-device transposes.
Per-head score matmuls (K=64) are row-packed in pairs into PE rows 0-63 /
64-127; attn@V matmuls (M=64) are col-packed in pairs.

Schedule (v2): attention groups are emitted at high priority with
score-matmul lookahead (score kt+1 before attn@V kt) so the sigmoid
stream on ScalarE is never blocked behind PE head-of-line waits;
projection work is emitted at low priority and dynamically fills PE idle
slots.  AV accumulation is pair-sequential (1 PSUM bank live).  Input
tile pool is sized to hold every input block (no WAR DMA stalls); the
warmup DCE-keeper DMA goes to a separate dummy output so it cannot block
the DMA queue.  Output rows are evacuated as whole [128,1024] DMAs, the
final tiles cast on Scalar+Vector in parallel.
"""

import numpy as np

BSZ, QLEN, KLEN = 2, 1024, 2048
HID = 1024
NH, HD = 16, 64
SCALE = 1.0 / (HD ** 0.5)
N_CORES = 8
GSLICE = 256           # hidden slice per core (4 heads = 2 head-pairs)
P = 128

_cache = {}


def _build(nkt):
    import concourse.bass as bass
    import concourse.tile as tile
    from concourse import bacc, mybir

    f32 = mybir.dt.float32
    f16 = mybir.dt.float16
    SIG = mybir.ActivationFunctionType.Sigmoid

    klen_c = nkt * P          # compacted + padded klen
    blocks = []
    pos = 0
    while pos < klen_c:
        blocks.append((pos, min(512, klen_c - pos)))
        pos += 512
    nblk = len(blocks)

    nc = bacc.Bacc("TRN2", target_bir_lowering=False, debug=False,
                   num_devices=N_CORES)

    # Pre-blocked inputs: x[blk, p, c, l] = x_T[c*128+p, blk*512+l].
    qT_v = nc.dram_tensor("qT", [2, P, HID // P, 512], f16, kind="ExternalInput").ap()
    kT_v = nc.dram_tensor("kT", [nblk, P, HID // P, 512], f16, kind="ExternalInput").ap()
    vT_v = nc.dram_tensor("vT", [nblk, P, HID // P, 512], f16, kind="ExternalInput").ap()
    wq_v = nc.dram_tensor("wq", [P, HID // P, GSLICE], f16, kind="ExternalInput").ap()
    wk_v = nc.dram_tensor("wk", [P, HID // P, GSLICE], f16, kind="ExternalInput").ap()
    wv_v = nc.dram_tensor("wv", [P, HID // P, GSLICE], f16, kind="ExternalInput").ap()
    wo_v = nc.dram_tensor("wo", [P, 2, HID], f16, kind="ExternalInput").ap()
    po_ap = nc.dram_tensor("po", [QLEN, HID], f16, kind="ExternalOutput").ap()
    dump_ap = nc.dram_tensor("dump", [1, 1], f16, kind="ExternalOutput").ap()

    NC_ = HID // P      # 8 contraction chunks

    with tile.TileContext(nc) as tc:
        with tc.tile_pool(name="sb", bufs=1) as sb, \
             tc.tile_pool(name="xin", bufs=2 * nblk + 2) as xin_pool, \
             tc.tile_pool(name="pt", bufs=4) as pt_pool, \
             tc.tile_pool(name="ost", bufs=4) as ost_pool, \
             tc.tile_pool(name="mm", bufs=2, space="PSUM") as mm_pool, \
             tc.tile_pool(name="av", bufs=2, space="PSUM") as av_pool, \
             tc.tile_pool(name="sps", bufs=2, space="PSUM") as s_pool:

            # ---- persistent tiles ----
            wq_sb = sb.tile([P, NC_, GSLICE], f16, tag="wq")
            wk_sb = sb.tile([P, NC_, GSLICE], f16, tag="wk")
            wv_sb = sb.tile([P, NC_, GSLICE], f16, tag="wv")
            wo_sb = sb.tile([P, 2, HID], f16, tag="wo")

            v_sb = sb.tile([P, nkt, GSLICE], f16, tag="v")      # V natural [klen_c, 256]
            kt_sb = sb.tile([P, 2, klen_c], f16, tag="kt")      # K^T [hd(2x128), klen_c]
            qt_sb = sb.tile([P, 2, QLEN], f16, tag="qt")        # Q^T [hd, qlen]
            avt_sb = sb.tile([P, 2, 2, 512], f16, tag="avt")    # AV^T [hd, pair, qc, q]

            xq_t, xk_t, xv_t = {}, {}, {}

            # ---- DMA issue (order = priority = arrival urgency) ----
            def dma_x(store, dram, blk, chunks, nm=""):
                x = xin_pool.tile([P, NC_, 512], f16, tag="xin",
                                  name=f"x{nm}{blk}")
                blen = blocks[blk][1] if dram is not qT_v else 512
                for cc in range(0, NC_, chunks):
                    nc.sync.dma_start(out=x[:, cc:cc + chunks, 0:blen],
                                      in_=dram[blk, :, cc:cc + chunks, 0:blen])
                store[blk] = x

            nc.sync.dma_start(out=wk_sb[:], in_=wk_v)
            nc.sync.dma_start(out=wq_sb[:], in_=wq_v)
            dma_x(xk_t, kT_v, 0, 2, "k")
            dma_x(xq_t, qT_v, 0, 2, "q")
            nc.sync.dma_start(out=wv_sb[:], in_=wv_v)
            dma_x(xv_t, vT_v, 0, 4, "v")
            if nblk > 1:
                dma_x(xk_t, kT_v, 1, 8, "k")
                dma_x(xv_t, vT_v, 1, 8, "v")
            dma_x(xq_t, qT_v, 1, 8, "q")
            for blk in range(2, nblk):
                dma_x(xk_t, kT_v, blk, 8, "k")
                dma_x(xv_t, vT_v, blk, 8, "v")
            nc.sync.dma_start(out=wo_sb[:], in_=wo_v)

            # ---- PE warm-up (keeps HAM at 2.4 GHz until real work) ----
            wtmp = sb.tile([P, 512], f16, tag="wtmp")
            nc.vector.memset(wtmp[:], 0.0)
            wps = mm_pool.tile([P, 512], f32, tag="mm", name="warm_ps")
            for _ in range(12):
                nc.tensor.matmul(wps[:], wtmp[:, 0:128], wtmp[:],
                                 start=True, stop=True)
            wsb = sb.tile([1, 1], f16, tag="wsb")
            nc.vector.tensor_copy(wsb[:], wps[0:1, 0:1])

            # ---- projection pieces ----
            def proj_k(blk):
                pos, blen = blocks[blk]
                xk = xk_t[blk]
                for half in range(2):
                    kps = mm_pool.tile([P, 512], f32, tag="mm",
                                       name=f"kps{blk}_{half}")
                    for c in range(NC_):
                        nc.tensor.matmul(
                            kps[:, 0:blen],
                            wk_sb[:, c, half * P:(half + 1) * P],
                            xk[:, c, 0:blen],
                            start=(c == 0), stop=(c == NC_ - 1),
                        )
                    nc.vector.tensor_copy(
                        kt_sb[:, half, pos:pos + blen], kps[:, 0:blen])

            def proj_q(qblk):
                xq = xq_t[qblk]
                for half in range(2):
                    qps = mm_pool.tile([P, 512], f32, tag="mm",
                                       name=f"qps{qblk}_{half}")
                    for c in range(NC_):
                        nc.tensor.matmul(
                            qps[:],
                            wq_sb[:, c, half * P:(half + 1) * P],
                            xq[:, c, :],
                            start=(c == 0), stop=(c == NC_ - 1),
                        )
                    nc.vector.tensor_copy(
                        qt_sb[:, half, qblk * 512:(qblk + 1) * 512], qps[:])

            def proj_v(blk, jj):
                """V projection for one pair of klen 128-tiles."""
                pos, blen = blocks[blk]
                ntile = blen // P
                xv = xv_t[blk]
                nj = min(2, ntile - jj * 2)
                if nj <= 0:
                    return
                vps = mm_pool.tile([P, 2, GSLICE], f32, tag="mm",
                                   name=f"vps{blk}_{jj}")
                for j in range(nj):
                    ktl = jj * 2 + j
                    for c in range(NC_):
                        nc.tensor.matmul(
                            vps[:, j, :],
                            xv[:, c, ktl * P:(ktl + 1) * P],
                            wv_sb[:, c, :],
                            start=(c == 0), stop=(c == NC_ - 1),
                        )
                kt0 = pos // P + jj * 2
                nc.vector.tensor_copy(v_sb[:, kt0:kt0 + nj, :], vps[:, 0:nj, :])

            # ---- attention ----
            av_tiles = {}

            def score(qc, pair, kt):
                sps = s_pool.tile([P, 2, 512], f32, tag="s",
                                  name=f"s{qc}_{pair}_{kt}")
                for h in range(2):
                    nc.tensor.matmul(
                        sps[:, h, :],
                        kt_sb[64 * h:64 * h + 64, pair, kt * P:(kt + 1) * P],
                        qt_sb[64 * h:64 * h + 64, pair, qc * 512:(qc + 1) * 512],
                        start=True, stop=True,
                    )
                return sps

            def sig_av(qc, pair, kt, sps):
                psb = pt_pool.tile([P, 2, 512], f16, tag="p",
                                   name=f"p{qc}_{pair}_{kt}")
                nc.scalar.activation(psb[:], sps[:], SIG, scale=float(SCALE))
                if (qc, pair) not in av_tiles:
                    av_tiles[(qc, pair)] = av_pool.tile(
                        [P, 512], f32, tag="av", name=f"av_{qc}_{pair}")
                avps = av_tiles[(qc, pair)]
                for h in range(2):
                    nc.tensor.matmul(
                        avps[64 * h:64 * h + 64, :],
                        v_sb[:, kt, pair * P + 64 * h: pair * P + 64 * h + 64],
                        psb[:, h, :],
                        start=(kt == 0), stop=(kt == nkt - 1),
                    )
                if kt == nkt - 1:
                    nc.vector.tensor_copy(avt_sb[:, pair, qc, :], avps[:])
                    del av_tiles[(qc, pair)]

            def out_proj(qc, qt, cast_split=False):
                ost = ost_pool.tile([P, 2, 512], f16, tag="ost",
                                    name=f"os{qc}_{qt}")
                for nn in range(2):
                    ops = mm_pool.tile([P, 512], f32, tag="mm",
                                       name=f"o{qc}_{qt}_{nn}")
                    for pr in range(2):
                        nc.tensor.matmul(
                            ops[:],
                            avt_sb[:, pr, qc, qt * P:(qt + 1) * P],
                            wo_sb[:, pr, nn * 512:(nn + 1) * 512],
                            start=(pr == 0), stop=(pr == 1),
                        )
                    if cast_split and nn == 1:
                        nc.scalar.copy(ost[:, nn, :], ops[:])
                    else:
                        nc.vector.tensor_copy(ost[:, nn, :], ops[:])
                r0 = qc * 512 + qt * P
                nc.sync.dma_start(out=po_ap[r0:r0 + P, :], in_=ost[:])

            # ---- emission (must be dataflow order: Tile derives deps
            # from program order).  Attention groups are interleaved with
            # projection "fill" thunks scheduled by deadline so the PE
            # always has independent work during sigmoid waits. ----
            proj_k(0)
            proj_q(0)
            for jj in range((blocks[0][1] // P + 1) // 2):
                proj_v(0, jj)

            # fill thunks for pass(0,0): (deadline_iter, fn).  score(kt+1)
            # is emitted during iter kt, so a block's proj_k deadline is
            # first_kt-2; proj_v covering kt has deadline kt-1.
            fills = []
            for blk in range(1, nblk):
                first_kt = blocks[blk][0] // P
                fills.append((first_kt - 2, lambda b=blk: proj_k(b)))
                for jj in range((blocks[blk][1] // P + 1) // 2):
                    kt0 = first_kt + jj * 2
                    fills.append((kt0 - 1, lambda b=blk, j=jj: proj_v(b, j)))
            fills.append((nkt - 1, lambda: proj_q(1)))
            fills.sort(key=lambda x: x[0])

            def attn_fill(qc, pair, fill_map):
                sps = score(qc, pair, 0)
                for kt in range(nkt):
                    cur = sps
                    if kt + 1 < nkt:
                        sps = score(qc, pair, kt + 1)
                    sig_av(qc, pair, kt, cur)
                    for fn in fill_map.get(kt, []):
                        fn()

            # pass (0,0): one fill thunk per iter, deadlines force earlier
            fm = {}
            nxt = 0
            for kt in range(nkt):
                while nxt < len(fills) and fills[nxt][0] <= kt:
                    fm.setdefault(kt, []).append(fills[nxt][1])
                    nxt += 1
                if kt not in fm and nxt < len(fills):
                    fm.setdefault(kt, []).append(fills[nxt][1])
                    nxt += 1
            while nxt < len(fills):
                fm.setdefault(nkt - 1, []).append(fills[nxt][1])
                nxt += 1
            attn_fill(0, 0, fm)
            attn_fill(0, 1, {})

            # pass (1,0): interleave out_proj(0) tiles
            fm10 = {}
            for qt in range(4):
                fm10.setdefault(min(2 * qt + 1, nkt - 1), []).append(
                    lambda q=qt: out_proj(0, q))
            attn_fill(1, 0, fm10)

            # pass (1,1): start the pr=0 half of out_proj(1, qt0/qt1)
            # early (avt(1,pair0) is ready); finish + evacuate in the tail.
            op1_ps = {}

            def op1_pr0(qt):
                for nn in range(2):
                    ops = mm_pool.tile([P, 512], f32, tag="mm",
                                       name=f"o1_{qt}_{nn}")
                    op1_ps[(qt, nn)] = ops
                    nc.tensor.matmul(
                        ops[:], avt_sb[:, 0, 1, qt * P:(qt + 1) * P],
                        wo_sb[:, 0, nn * 512:(nn + 1) * 512],
                        start=True, stop=False,
                    )

            attn_fill(1, 1, {min(3, nkt - 2): [lambda: op1_pr0(0)],
                             min(5, nkt - 1): [lambda: op1_pr0(1)]})

            # ---- tail ----
            for qt in range(2):
                ost = ost_pool.tile([P, 2, 512], f16, tag="ost",
                                    name=f"os1_{qt}")
                for nn in range(2):
                    ops = op1_ps[(qt, nn)]
                    nc.tensor.matmul(
                        ops[:], avt_sb[:, 1, 1, qt * P:(qt + 1) * P],
                        wo_sb[:, 1, nn * 512:(nn + 1) * 512],
                        start=False, stop=True,
                    )
                    if nn == 1:
                        nc.scalar.copy(ost[:, nn, :], ops[:])
                    else:
                        nc.vector.tensor_copy(ost[:, nn, :], ops[:])
                r0 = 512 + qt * P
                nc.sync.dma_start(out=po_ap[r0:r0 + P, :], in_=ost[:])
            out_proj(1, 2, cast_split=True)
            out_proj(1, 3, cast_split=True)

            # warmup DCE keeper - separate output, cannot block po DMAs
            nc.sync.dma_start(out=dump_ap[0:1, 0:1], in_=wsb[:])

    nc.compile()
    return nc


def _prep_in_maps(query, key, value, attn_mask, Wq, Wk, Wv, Wo):
    query = np.asarray(query, np.float32)
    key = np.asarray(key, np.float32)
    value = np.asarray(value, np.float32)
    mask = np.asarray(attn_mask)
    Wq = np.asarray(Wq, np.float32)
    Wk = np.asarray(Wk, np.float32)
    Wv = np.asarray(Wv, np.float32)
    Wo = np.asarray(Wo, np.float32)

    # Masked klen positions contribute exactly 0 (reference: sigmoid(-1e30)
    # == 0), so compact each batch to its unmasked positions, zero-padded
    # to a common multiple of 128.
    idxs = [np.nonzero(mask[b] != 0)[0] for b in range(BSZ)]
    klen_eff = max(len(ix) for ix in idxs)
    nkt = max(4, -(-klen_eff // P))
    klen_c = nkt * P

    nblk = (klen_c + 511) // 512
    klen_pad = nblk * 512

    def block_x(xT, width, pad_to):
        # [HID, width] -> [nblocks, 128, 8, 512] contiguous, zero-padded
        full = np.zeros((HID, pad_to), np.float16)
        full[:, :width] = xT
        nb = pad_to // 512
        return np.ascontiguousarray(
            full.reshape(HID // P, P, nb, 512).transpose(2, 1, 0, 3))

    kTc, vTc = [], []
    for b in range(BSZ):
        ix = idxs[b]
        kTc.append(block_x(key[b].T[:, ix].astype(np.float16), len(ix), klen_pad))
        vTc.append(block_x(value[b].T[:, ix].astype(np.float16), len(ix), klen_pad))

    qT0 = {}
    in_maps = []
    for core in range(N_CORES):
        b, g = divmod(core, 4)
        sl = slice(g * GSLICE, (g + 1) * GSLICE)
        if b not in qT0:
            qT0[b] = block_x(query[b].T.astype(np.float16), QLEN, QLEN)
        in_maps.append({
            "qT": qT0[b],
            "kT": kTc[b],
            "vT": vTc[b],
            "wq": np.ascontiguousarray(
                Wq[:, sl].astype(np.float16).reshape(HID // P, P, GSLICE)
                .transpose(1, 0, 2)),
            "wk": np.ascontiguousarray(
                Wk[:, sl].astype(np.float16).reshape(HID // P, P, GSLICE)
                .transpose(1, 0, 2)),
            "wv": np.ascontiguousarray(
                Wv[:, sl].astype(np.float16).reshape(HID // P, P, GSLICE)
                .transpose(1, 0, 2)),
            "wo": np.ascontiguousarray(
                Wo[sl, :].astype(np.float16).reshape(2, P, HID)
                .transpose(1, 0, 2)),
        })
    return in_maps, nkt


def _run(in_maps, nkt, trace):
    from concourse.bass_utils import run_bass_kernel_spmd

    if nkt not in _cache:
        _cache[nkt] = _build(nkt)
    res = run_bass_kernel_spmd(_cache[nkt], in_maps, list(range(N_CORES)),
                               trace=trace)
    out = np.zeros((BSZ, QLEN, HID), np.float32)
    for core in range(N_CORES):
        out[core // 4] += res.results[core]["po"].astype(np.float32)
    return out, res


def kernel(query, key, value, attn_mask, Wq, Wk, Wv, Wo):
    in_maps, nkt = _prep_in_maps(query, key, value, attn_mask, Wq, Wk, Wv, Wo)
    out, _ = _run(in_maps, nkt, trace=False)
    return out


def run_traced(query, key, value, attn_mask, Wq, Wk, Wv, Wo):
    """Like kernel() but with NTFF profiling; returns (out, exec_time_ns)."""
    in_maps, nkt = _prep_in_maps(query, key, value, attn_mask, Wq, Wk, Wv, Wo)
    out, res = _run(in_maps, nkt, trace=True)
    return out, res.exec_time_ns


# revision 21
# speedup vs baseline: 1.0669x; 1.0669x over previous
"""TRN2 Bass kernel for nn_MultiHeadAttn_1580547971654.

Multi-head attention with sigmoid activation (no softmax normalization),
2D key-side mask. query [2,1024,1024], key/value [2,2048,1024],
Wq/Wk/Wv [1024,1024], Wo [1024,1024], NH=16, HD=64.

Sharding (8 cores): data-parallel over batch (2) x tensor-parallel over
head groups (4 groups of 4 heads).  Core (b, g) computes
  partial[b] = sigmoid(scale * (q[b] Wq[:,G]) (k[b] Wk[:,G])^T) ((v[b]*mask) Wv[:,G]) Wo[G,:]
with G = head-group g's 256-wide hidden slice.  Host sums 4 partials per
batch.

Mask compaction: masked klen positions contribute exactly zero
(reference: sigmoid(-1e30) == 0), so the host gathers only unmasked
key/value columns, zero-padded to a multiple of 128.  With the uniform
0/1 mask this halves the klen-side work exactly.

Numerics: fp16 operands everywhere (TRN2 PE does native fp16 multiplies
with fp32 PSUM accumulation), so the only error is rounding tensors to
fp16 (2^-11).  Scale is folded into the sigmoid activation's scale.

Layout: activations are uploaded pre-transposed ([hidden, len]) so all
matmuls contract over the partition axis with no on-device transposes.
Per-head score matmuls (K=64) are row-packed in pairs into PE rows 0-63 /
64-127; attn@V matmuls (M=64) are col-packed in pairs.

Schedule (v2): attention groups are emitted at high priority with
score-matmul lookahead (score kt+1 before attn@V kt) so the sigmoid
stream on ScalarE is never blocked behind PE head-of-line waits;
projection work is emitted at low priority and dynamically fills PE idle
slots.  AV accumulation is pair-sequential (1 PSUM bank live).  Input
tile pool is sized to hold every input block (no WAR DMA stalls); the
warmup DCE-keeper DMA goes to a separate dummy output so it cannot block
the DMA queue.  Output rows are evacuated as whole [128,1024] DMAs, the
final tiles cast on Scalar+Vector in parallel.
"""

import numpy as np

BSZ, QLEN, KLEN = 2, 1024, 2048
HID = 1024
NH, HD = 16, 64
SCALE = 1.0 / (HD ** 0.5)
N_CORES = 8
GSLICE = 256           # hidden slice per core (4 heads = 2 head-pairs)
P = 128

_cache = {}


def _build(nkt):
    import concourse.bass as bass
    import concourse.tile as tile
    from concourse import bacc, mybir

    f32 = mybir.dt.float32
    f16 = mybir.dt.float16
    SIG = mybir.ActivationFunctionType.Sigmoid

    klen_c = nkt * P          # compacted + padded klen
    blocks = []
    pos = 0
    while pos < klen_c:
        blocks.append((pos, min(512, klen_c - pos)))
        pos += 512
    nblk = len(blocks)

    nc = bacc.Bacc("TRN2", target_bir_lowering=False, debug=False,
                   num_devices=N_CORES)

    # Pre-blocked inputs: x[blk, p, c, l] = x_T[c*128+p, blk*512+l].
    qT_v = nc.dram_tensor("qT", [2, P, HID // P, 512], f16, kind="ExternalInput").ap()
    kT_v = nc.dram_tensor("kT", [nblk, P, HID // P, 512], f16, kind="ExternalInput").ap()
    vT_v = nc.dram_tensor("vT", [nblk, P, HID // P, 512], f16, kind="ExternalInput").ap()
    wq_v = nc.dram_tensor("wq", [P, HID // P, GSLICE], f16, kind="ExternalInput").ap()
    wk_v = nc.dram_tensor("wk", [P, HID // P, GSLICE], f16, kind="ExternalInput").ap()
    wv_v = nc.dram_tensor("wv", [P, HID // P, GSLICE], f16, kind="ExternalInput").ap()
    wo_v = nc.dram_tensor("wo", [P, 2, HID], f16, kind="ExternalInput").ap()
    po_ap = nc.dram_tensor("po", [QLEN, HID], f16, kind="ExternalOutput").ap()
    dump_ap = nc.dram_tensor("dump", [1, 1], f16, kind="ExternalOutput").ap()

    NC_ = HID // P      # 8 contraction chunks

    with tile.TileContext(nc) as tc:
        with tc.tile_pool(name="sb", bufs=1) as sb, \
             tc.tile_pool(name="xin", bufs=2 * nblk + 2) as xin_pool, \
             tc.tile_pool(name="pt", bufs=14) as pt_pool, \
             tc.tile_pool(name="ost", bufs=4) as ost_pool, \
             tc.tile_pool(name="mm", bufs=2, space="PSUM") as mm_pool, \
             tc.tile_pool(name="av", bufs=2, space="PSUM") as av_pool, \
             tc.tile_pool(name="sps", bufs=2, space="PSUM") as s_pool:

            # ---- persistent tiles ----
            wq_sb = sb.tile([P, NC_, GSLICE], f16, tag="wq")
            wk_sb = sb.tile([P, NC_, GSLICE], f16, tag="wk")
            wv_sb = sb.tile([P, NC_, GSLICE], f16, tag="wv")
            wo_sb = sb.tile([P, 2, HID], f16, tag="wo")

            v_sb = sb.tile([P, nkt, GSLICE], f16, tag="v")      # V natural [klen_c, 256]
            kt_sb = sb.tile([P, 2, klen_c], f16, tag="kt")      # K^T [hd(2x128), klen_c]
            qt_sb = sb.tile([P, 2, QLEN], f16, tag="qt")        # Q^T [hd, qlen]
            avt_sb = sb.tile([P, 2, 2, 512], f16, tag="avt")    # AV^T [hd, pair, qc, q]

            xq_t, xk_t, xv_t = {}, {}, {}

            # ---- DMA issue (order = priority = arrival urgency) ----
            def dma_x(store, dram, blk, chunks, nm=""):
                x = xin_pool.tile([P, NC_, 512], f16, tag="xin",
                                  name=f"x{nm}{blk}")
                blen = blocks[blk][1] if dram is not qT_v else 512
                for cc in range(0, NC_, chunks):
                    nc.sync.dma_start(out=x[:, cc:cc + chunks, 0:blen],
                                      in_=dram[blk, :, cc:cc + chunks, 0:blen])
                store[blk] = x

            nc.sync.dma_start(out=wk_sb[:], in_=wk_v)
            nc.sync.dma_start(out=wq_sb[:], in_=wq_v)
            dma_x(xk_t, kT_v, 0, 2, "k")
            dma_x(xq_t, qT_v, 0, 2, "q")
            for blk in range(1, nblk):
                dma_x(xk_t, kT_v, blk, 8, "k")
            dma_x(xq_t, qT_v, 1, 8, "q")
            nc.sync.dma_start(out=wv_sb[:], in_=wv_v)
            dma_x(xv_t, vT_v, 0, 4, "v")
            for blk in range(1, nblk):
                dma_x(xv_t, vT_v, blk, 8, "v")
            nc.sync.dma_start(out=wo_sb[:], in_=wo_v)

            # ---- PE warm-up (keeps HAM at 2.4 GHz until real work) ----
            wtmp = sb.tile([P, 512], f16, tag="wtmp")
            nc.vector.memset(wtmp[:], 0.0)
            wps = mm_pool.tile([P, 512], f32, tag="mm", name="warm_ps")
            for _ in range(12):
                nc.tensor.matmul(wps[:], wtmp[:, 0:128], wtmp[:],
                                 start=True, stop=True)
            wsb = sb.tile([1, 1], f16, tag="wsb")
            nc.vector.tensor_copy(wsb[:], wps[0:1, 0:1])

            # ---- projection pieces ----
            def proj_k(blk):
                pos, blen = blocks[blk]
                xk = xk_t[blk]
                for half in range(2):
                    kps = mm_pool.tile([P, 512], f32, tag="mm",
                                       name=f"kps{blk}_{half}")
                    for c in range(NC_):
                        nc.tensor.matmul(
                            kps[:, 0:blen],
                            wk_sb[:, c, half * P:(half + 1) * P],
                            xk[:, c, 0:blen],
                            start=(c == 0), stop=(c == NC_ - 1),
                        )
                    nc.vector.tensor_copy(
                        kt_sb[:, half, pos:pos + blen], kps[:, 0:blen])

            def proj_q(qblk):
                xq = xq_t[qblk]
                for half in range(2):
                    qps = mm_pool.tile([P, 512], f32, tag="mm",
                                       name=f"qps{qblk}_{half}")
                    for c in range(NC_):
                        nc.tensor.matmul(
                            qps[:],
                            wq_sb[:, c, half * P:(half + 1) * P],
                            xq[:, c, :],
                            start=(c == 0), stop=(c == NC_ - 1),
                        )
                    nc.vector.tensor_copy(
                        qt_sb[:, half, qblk * 512:(qblk + 1) * 512], qps[:])

            def proj_v(blk, jj):
                """V projection for one pair of klen 128-tiles."""
                pos, blen = blocks[blk]
                ntile = blen // P
                xv = xv_t[blk]
                nj = min(2, ntile - jj * 2)
                if nj <= 0:
                    return
                vps = mm_pool.tile([P, 2, GSLICE], f32, tag="mm",
                                   name=f"vps{blk}_{jj}")
                for j in range(nj):
                    ktl = jj * 2 + j
                    for c in range(NC_):
                        nc.tensor.matmul(
                            vps[:, j, :],
                            xv[:, c, ktl * P:(ktl + 1) * P],
                            wv_sb[:, c, :],
                            start=(c == 0), stop=(c == NC_ - 1),
                        )
                kt0 = pos // P + jj * 2
                nc.vector.tensor_copy(v_sb[:, kt0:kt0 + nj, :], vps[:, 0:nj, :])

            # ---- attention primitives ----
            av_tiles = {}

            def score(qc, pair, kt):
                sps = s_pool.tile([P, 2, 512], f32, tag="s",
                                  name=f"s{qc}_{pair}_{kt}")
                for h in range(2):
                    nc.tensor.matmul(
                        sps[:, h, :],
                        kt_sb[64 * h:64 * h + 64, pair, kt * P:(kt + 1) * P],
                        qt_sb[64 * h:64 * h + 64, pair, qc * 512:(qc + 1) * 512],
                        start=True, stop=True,
                    )
                return sps

            def sig(qc, pair, kt, sps):
                psb = pt_pool.tile([P, 2, 512], f16, tag="p",
                                   name=f"p{qc}_{pair}_{kt}")
                nc.scalar.activation(psb[:], sps[:], SIG, scale=float(SCALE))
                return psb

            def av(qc, pair, kt, psb):
                if (qc, pair) not in av_tiles:
                    av_tiles[(qc, pair)] = av_pool.tile(
                        [P, 512], f32, tag="av", name=f"av_{qc}_{pair}")
                avps = av_tiles[(qc, pair)]
                for h in range(2):
                    nc.tensor.matmul(
                        avps[64 * h:64 * h + 64, :],
                        v_sb[:, kt, pair * P + 64 * h: pair * P + 64 * h + 64],
                        psb[:, h, :],
                        start=(kt == 0), stop=(kt == nkt - 1),
                    )
                if kt == nkt - 1:
                    nc.vector.tensor_copy(avt_sb[:, pair, qc, :], avps[:])
                    del av_tiles[(qc, pair)]

            def op_nn(qc, qt, nn, ost_box, cast_eng):
                if '' not in ost_box:
                    ost_box[''] = ost_pool.tile([P, 2, 512], f16, tag="ost",
                                                name=f"os{qc}_{qt}")
                ost = ost_box['']
                ops = mm_pool.tile([P, 512], f32, tag="mm",
                                   name=f"o{qc}_{qt}_{nn}")
                for pr in range(2):
                    nc.tensor.matmul(
                        ops[:],
                        avt_sb[:, pr, qc, qt * P:(qt + 1) * P],
                        wo_sb[:, pr, nn * 512:(nn + 1) * 512],
                        start=(pr == 0), stop=(pr == 1),
                    )
                if cast_eng == 's':
                    nc.scalar.copy(ost[:, nn, :], ops[:])
                else:
                    nc.vector.tensor_copy(ost[:, nn, :], ops[:])
                if nn == 1:
                    r0 = qc * 512 + qt * P
                    nc.sync.dma_start(out=po_ap[r0:r0 + P, :], in_=ost[:])

            # ---- slot-based emission ----
            # Emission order defines both dataflow dependencies and the
            # Tile scheduler's priorities, and each engine executes its
            # stream IN ORDER; so emission must match expected runtime
            # readiness.  The sigmoid stream (score lookahead + sigmoid,
            # one group per slot) is the spine; all other PE work is cut
            # into small atoms with a release slot (when its input DMA
            # should have landed) and an optional deadline slot (when a
            # later spine op needs it).
            passes = [(0, 0), (0, 1), (1, 0), (1, 1)]
            slots = [(qc, pair, kt) for qc, pair in passes
                     for kt in range(nkt)]
            nslots = len(slots)

            def slot_of(qc, pair, kt):
                return passes.index((qc, pair)) * nkt + kt

            # release-slot estimate for a DMA that lands at time T (us):
            # slot 0 fires around t=20us, ~1.3us per slot.
            def rslot(t_us):
                return max(0, int((t_us - 20.0) / 1.3) + 1)

            # estimated arrival times (serial DMA issue at ~0.35us per
            # dma_start + ~2.86us/MB transfer, from t=6.6us), matching
            # the dma emission order above.
            t_arr = {}
            _t = [6.6]

            def land(name, size_mb, n_instr=1):
                _t[0] += 0.35 * n_instr + size_mb * 2.86
                t_arr[name] = _t[0]
            land('wk', .5)
            land('wq', .5)
            land('xk0', 1., 4)
            land('xq0', 1., 4)
            for blk in range(1, nblk):
                land(f'xk{blk}', blocks[blk][1] / 512.)
            land('xq1', 1.)
            land('wv', .5)
            land('xv0', 1., 2)
            for blk in range(1, nblk):
                land(f'xv{blk}', blocks[blk][1] / 512.)
            land('wo', .5)

            queue = []  # list of [release_slot, deadline_slot|None, fn]

            def pump(g, budget=2):
                emitted = 0
                i = 0
                while i < len(queue):
                    rel, dl, fn = queue[i]
                    if (dl is not None and dl <= g) or \
                       (emitted < budget and rel <= g):
                        fn()
                        queue.pop(i)
                        emitted += 1
                        continue
                    i += 1

            # projection atoms -> queue
            def k_atoms(blk):
                first_kt = blocks[blk][0] // P
                dl = max(0, first_kt - 2)
                rel = rslot(t_arr[f'xk{blk}'])
                pos, blen = blocks[blk]
                for half in range(2):
                    box = {}
                    def a1(blk=blk, half=half, box=box, blen=blen):
                        kps = mm_pool.tile([P, 512], f32, tag="mm",
                                           name=f"kps{blk}_{half}")
                        box['t'] = kps
                        for c in range(4):
                            nc.tensor.matmul(
                                kps[:, 0:blen],
                                wk_sb[:, c, half * P:(half + 1) * P],
                                xk_t[blk][:, c, 0:blen],
                                start=(c == 0), stop=False)
                    def a2(blk=blk, half=half, box=box, pos=pos, blen=blen):
                        kps = box['t']
                        for c in range(4, NC_):
                            nc.tensor.matmul(
                                kps[:, 0:blen],
                                wk_sb[:, c, half * P:(half + 1) * P],
                                xk_t[blk][:, c, 0:blen],
                                start=False, stop=(c == NC_ - 1))
                        nc.vector.tensor_copy(
                            kt_sb[:, half, pos:pos + blen], kps[:, 0:blen])
                    queue.append([rel, dl, a1])
                    queue.append([rel, dl, a2])

            def q_atoms(qblk):
                dl = slot_of(1, 0, 0) - 2 if qblk == 1 else 0
                rel = rslot(t_arr[f'xq{qblk}'])
                for half in range(2):
                    box = {}
                    def a1(qblk=qblk, half=half, box=box):
                        qps = mm_pool.tile([P, 512], f32, tag="mm",
                                           name=f"qps{qblk}_{half}")
                        box['t'] = qps
                        for c in range(4):
                            nc.tensor.matmul(
                                qps[:],
                                wq_sb[:, c, half * P:(half + 1) * P],
                                xq_t[qblk][:, c, :],
                                start=(c == 0), stop=False)
                    def a2(qblk=qblk, half=half, box=box):
                        qps = box['t']
                        for c in range(4, NC_):
                            nc.tensor.matmul(
                                qps[:],
                                wq_sb[:, c, half * P:(half + 1) * P],
                                xq_t[qblk][:, c, :],
                                start=False, stop=(c == NC_ - 1))
                        nc.vector.tensor_copy(
                            qt_sb[:, half, qblk * 512:(qblk + 1) * 512],
                            qps[:])
                    queue.append([rel, dl, a1])
                    queue.append([rel, dl, a2])

            def v_atoms(blk):
                rel = rslot(t_arr[f'xv{blk}'])
                pos, blen = blocks[blk]
                for j in range(blen // P):
                    kt_g = pos // P + j
                    # deadline: just before its first AV atom could pop
                    def aj(blk=blk, j=j, kt_g=kt_g):
                        vps = mm_pool.tile([P, GSLICE], f32, tag="mm",
                                           name=f"vps{blk}_{j}")
                        for c in range(NC_):
                            nc.tensor.matmul(
                                vps[:],
                                xv_t[blk][:, c, j * P:(j + 1) * P],
                                wv_sb[:, c, :],
                                start=(c == 0), stop=(c == NC_ - 1))
                        nc.vector.tensor_copy(v_sb[:, kt_g, :], vps[:])
                    queue.append([rel, None, aj])

            # ---- spine ----
            # block 0 K/Q projections run before the spine (they gate it)
            proj_k(0)
            proj_q(0)
            for blk in range(1, nblk):
                k_atoms(blk)
            v_atoms(0)
            q_atoms(1)
            for blk in range(1, nblk):
                v_atoms(blk)

            op1_ps = {}
            sps_cur = score(*slots[0])
            for g, (qc, pair, kt) in enumerate(slots):
                psb = sig(qc, pair, kt, sps_cur)
                if g + 1 < nslots:
                    sps_cur = score(*slots[g + 1])
                # AV for this group becomes an atom; its release accounts
                # for the arrival + projection of the V block holding kt
                vblk = kt * P // 512
                first_kt = blocks[vblk][0] // P
                rel = max(g + 1,
                          rslot(t_arr[f'xv{vblk}']) + 1 + (kt - first_kt) // 2)
                queue.append([rel, None,
                              lambda qc=qc, pair=pair, kt=kt, psb=psb:
                              av(qc, pair, kt, psb)])
                if (qc, pair) == (0, 1) and kt == nkt - 1:
                    # out_proj(0) atoms follow avt(0,1) in queue order
                    for qt in range(4):
                        box = {}
                        for nn in range(2):
                            queue.append([g + 2 + qt, None,
                                          lambda qt=qt, nn=nn, box=box:
                                          op_nn(0, qt, nn, box, 'v')])
                if (qc, pair) == (1, 0) and kt == nkt - 1:
                    # pr=0 half of out_proj(1, qt=0): both mm slots are
                    # free by then (all projections/op(0) done)
                    def pr0():
                        for nn in range(2):
                            ops = mm_pool.tile([P, 512], f32, tag="mm",
                                               name=f"o1_0_{nn}")
                            op1_ps[nn] = ops
                            nc.tensor.matmul(
                                ops[:], avt_sb[:, 0, 1, 0:P],
                                wo_sb[:, 0, nn * 512:(nn + 1) * 512],
                                start=True, stop=False)
                    queue.append([g + 4, None, pr0])
                pump(g)

            # ---- tail: drain queue, finish out_proj(1) ----
            pump(nslots + 100, budget=len(queue) + 1)
            ost = ost_pool.tile([P, 2, 512], f16, tag="ost", name="os1_0")
            for nn in range(2):
                ops = op1_ps[nn]
                nc.tensor.matmul(
                    ops[:], avt_sb[:, 1, 1, 0:P],
                    wo_sb[:, 1, nn * 512:(nn + 1) * 512],
                    start=False, stop=True)
                if nn == 0:
                    nc.vector.tensor_copy(ost[:, nn, :], ops[:])
                else:
                    nc.scalar.copy(ost[:, nn, :], ops[:])
            nc.sync.dma_start(out=po_ap[512:512 + P, :], in_=ost[:])
            for qt in range(1, 4):
                box = {}
                op_nn(1, qt, 0, box, 'v')
                op_nn(1, qt, 1, box, 's')

            # warmup DCE keeper - separate output, cannot block po DMAs
            nc.sync.dma_start(out=dump_ap[0:1, 0:1], in_=wsb[:])

    nc.compile()
    return nc


def _prep_in_maps(query, key, value, attn_mask, Wq, Wk, Wv, Wo):
    query = np.asarray(query, np.float32)
    key = np.asarray(key, np.float32)
    value = np.asarray(value, np.float32)
    mask = np.asarray(attn_mask)
    Wq = np.asarray(Wq, np.float32)
    Wk = np.asarray(Wk, np.float32)
    Wv = np.asarray(Wv, np.float32)
    Wo = np.asarray(Wo, np.float32)

    # Masked klen positions contribute exactly 0 (reference: sigmoid(-1e30)
    # == 0), so compact each batch to its unmasked positions, zero-padded
    # to a common multiple of 128.
    idxs = [np.nonzero(mask[b] != 0)[0] for b in range(BSZ)]
    klen_eff = max(len(ix) for ix in idxs)
    nkt = max(4, -(-klen_eff // P))
    klen_c = nkt * P

    nblk = (klen_c + 511) // 512
    klen_pad = nblk * 512

    def block_x(xT, width, pad_to):
        # [HID, width] -> [nblocks, 128, 8, 512] contiguous, zero-padded
        full = np.zeros((HID, pad_to), np.float16)
        full[:, :width] = xT
        nb = pad_to // 512
        return np.ascontiguousarray(
            full.reshape(HID // P, P, nb, 512).transpose(2, 1, 0, 3))

    kTc, vTc = [], []
    for b in range(BSZ):
        ix = idxs[b]
        kTc.append(block_x(key[b].T[:, ix].astype(np.float16), len(ix), klen_pad))
        vTc.append(block_x(value[b].T[:, ix].astype(np.float16), len(ix), klen_pad))

    qT0 = {}
    in_maps = []
    for core in range(N_CORES):
        b, g = divmod(core, 4)
        sl = slice(g * GSLICE, (g + 1) * GSLICE)
        if b not in qT0:
            qT0[b] = block_x(query[b].T.astype(np.float16), QLEN, QLEN)
        in_maps.append({
            "qT": qT0[b],
            "kT": kTc[b],
            "vT": vTc[b],
            "wq": np.ascontiguousarray(
                Wq[:, sl].astype(np.float16).reshape(HID // P, P, GSLICE)
                .transpose(1, 0, 2)),
            "wk": np.ascontiguousarray(
                Wk[:, sl].astype(np.float16).reshape(HID // P, P, GSLICE)
                .transpose(1, 0, 2)),
            "wv": np.ascontiguousarray(
                Wv[:, sl].astype(np.float16).reshape(HID // P, P, GSLICE)
                .transpose(1, 0, 2)),
            "wo": np.ascontiguousarray(
                Wo[sl, :].astype(np.float16).reshape(2, P, HID)
                .transpose(1, 0, 2)),
        })
    return in_maps, nkt


def _run(in_maps, nkt, trace):
    from concourse.bass_utils import run_bass_kernel_spmd

    if nkt not in _cache:
        _cache[nkt] = _build(nkt)
    res = run_bass_kernel_spmd(_cache[nkt], in_maps, list(range(N_CORES)),
                               trace=trace)
    out = np.zeros((BSZ, QLEN, HID), np.float32)
    for core in range(N_CORES):
        out[core // 4] += res.results[core]["po"].astype(np.float32)
    return out, res


def kernel(query, key, value, attn_mask, Wq, Wk, Wv, Wo):
    in_maps, nkt = _prep_in_maps(query, key, value, attn_mask, Wq, Wk, Wv, Wo)
    out, _ = _run(in_maps, nkt, trace=False)
    return out


def run_traced(query, key, value, attn_mask, Wq, Wk, Wv, Wo):
    """Like kernel() but with NTFF profiling; returns (out, exec_time_ns)."""
    in_maps, nkt = _prep_in_maps(query, key, value, attn_mask, Wq, Wk, Wv, Wo)
    out, res = _run(in_maps, nkt, trace=True)
    return out, res.exec_time_ns
